# revision 32
# baseline (speedup 1.0000x reference)
"""Trainium2 Bass kernel for the NeuralODE problem.

Math (matching reference.py):
    20 Euler steps (10 segments x 2 steps, uniform dt => step size hi = 0.05):
        z_{i+1} = z_i + hi * ( tanh(z_i @ W1 + b1 + t_i*wt) @ W2 + b2 )

Device-side reformulation (per core, batch shard B=64):
    - Fold hi into W2:  W2' = hi * W2, c = hi * b2.
    - Keep the "state without accumulated c":  z'_i = z_i - i*c, so
        z'_{i+1} = z'_i + tanh(z'_i @ W1 + bias_i) @ W2'
      with bias_i = b1 + t_i*wt + i*(c @ W1)   (precomputed on host).
      Final output: z_20 = z'_20 + 20*c       (added on host).
    - State kept transposed (d-major) as zt[p, 64k+b] = z'[b, 128k+p] so it can be
      the stationary (lhsT) operand of orientation-B matmuls.
    - Both matmuls stream the (SBUF-resident) weights as the moving operand with
      N=512 chunks; the 64-wide batch stationary only fills half the PE columns,
      so two chunks run concurrently via tile_position col-tiling (0,0)/(0,64).
    - The per-step bias enters PSUM first through a K=1 ones-vector matmul.
    - Layout flips (batch-major PSUM result -> d/hid-major stationary for the next
      matmul) are done with full-128 PE transpose-mode matmuls against identity;
      one 128x128 transpose covers one 128-col block of both concurrent chunks.

Precision: matmul operands are float16 (1 cyc/col on the PE vs 4 for fp32;
fp32r would also be 1 cyc/col but its ISA rules require dst start_partition==0,
which forbids the (0,64) col-tiling). The master state zt stays fp32 in SBUF
and is re-cast to fp16 each step for the stationary operand; PSUM accumulation
is fp32 throughout, so only operand rounding (~5e-4) enters per step.

Sharding: pure data-parallel over batch (512 -> 8 x 64); weights replicated.
"""

import numpy as np

BS, D, HID = 512, 1024, 2048
NCORES = 8
B = BS // NCORES  # 64
NSTEP = 20
KD = D // 128  # 8 k-tiles for the D contraction
KH = HID // 128  # 16 k-tiles for the HID contraction
F32 = np.float32

MM_DTYPE = "float16"  # matmul operand dtype ("float16" | "bfloat16" | "float32")


def _np_dt(mm_dtype):
    return {"float16": np.float16, "bfloat16": None, "float32": np.float32}[mm_dtype]


def _build_program(mm_dtype=MM_DTYPE, repeat=1):
    import concourse.mybir as mybir
    from concourse import bacc
    from concourse.tile import TileContext

    nc = bacc.Bacc()
    f32 = mybir.dt.float32
    mmdt = getattr(mybir.dt, mm_dtype)
    TANH = mybir.ActivationFunctionType.Tanh

    zt_in = nc.dram_tensor("zt_in", [128, KD * B], f32, kind="ExternalInput")
    w1_d = nc.dram_tensor("w1", [128, KD * HID], mmdt, kind="ExternalInput")
    w2_d = nc.dram_tensor("w2", [128, KH * D], mmdt, kind="ExternalInput")
    # bias rows for the K=1 ones-matmul PSUM init, all steps
    brow_d = nc.dram_tensor("brow", [1, NSTEP * HID], mmdt, kind="ExternalInput")
    ones_d = nc.dram_tensor("ones", [1, B], mmdt, kind="ExternalInput")
    ident_d = nc.dram_tensor("ident", [128, 128], mmdt, kind="ExternalInput")
    zt_out = nc.dram_tensor("zt_out", [128, KD * B], f32, kind="ExternalOutput")

    # mm consumption order matches per-128-block production of the transposed
    # operand (block u of a PE transpose yields k-slices {u, u+4}).
    K_ORDER = [0, 4, 1, 5, 2, 6, 3, 7]

    with (
        TileContext(nc) as tc,
        tc.tile_pool(name="const", bufs=1) as cpool,
        tc.tile_pool(name="weights", bufs=1) as wpool,
        tc.tile_pool(name="state", bufs=1) as spool,
        tc.tile_pool(name="work", bufs=2) as hpool,
        tc.tile_pool(name="psumh", bufs=2, space="PSUM") as ph_pool,
        tc.tile_pool(name="psumt", bufs=2, space="PSUM") as pt_pool,
        tc.tile_pool(name="psumf", bufs=2, space="PSUM") as pf_pool,
    ):
        ident_sb = cpool.tile([128, 128], mmdt, tag="ident")
        nc.sync.dma_start(ident_sb[:], ident_d[:])
        ones_sb = cpool.tile([1, B], mmdt, tag="ones")
        nc.sync.dma_start(ones_sb[:], ones_d[:])
        # all 20 g1-bias rows in one small upfront DMA (before the weights on
        # the same HWDGE FIFO)
        brow_sb = cpool.tile([1, NSTEP * HID], mmdt, tag="brow")
        nc.sync.dma_start(brow_sb[:], brow_d[:])

        zt = spool.tile([128, KD * B], f32, tag="zt")  # master z'_T  [128, 512] fp32
        nc.sync.dma_start(zt[:], zt_in[:])
        zt_h = spool.tile([128, KD * B], mmdt, tag="zt_h")  # fp16 copy for the PE
        hT = spool.tile([128, KH * B], mmdt, tag="hT")  # tanh'd h, hid-major [128,1024]

        # per-k weight tiles, DMA-issued in mm consumption order so step-0
        # matmuls can start as soon as their own k-slice has landed
        w1t = [None] * KD
        for k in K_ORDER:
            w = wpool.tile([128, HID], mmdt, tag=f"w1_{k}")
            nc.sync.dma_start(w[:], w1_d[:, k * HID : (k + 1) * HID])
            w1t[k] = w
        w2t = []
        for k in range(KH):
            w = wpool.tile([128, D], mmdt, tag=f"w2_{k}")
            nc.sync.dma_start(w[:], w2_d[:, k * D : (k + 1) * D])
            w2t.append(w)

        def scan_body(_iv=None):
            for i in range(NSTEP):
                if i == 0:
                    nc.vector.tensor_copy(zt_h[:], zt[:])

                # ---- mm1: h_pre = z @ W1 + bias_i (K=1 ones mm inits PSUM) ----
                phs = []
                for g in range(2):
                    ph = ph_pool.tile([128, 512], f32, tag="ph")
                    phs.append(ph)
                    for half in range(2):
                        c = 2 * g + half
                        nc.tensor.matmul(
                            ph[64 * half : 64 * half + 64, :],
                            ones_sb[:1, :],
                            brow_sb[:1, i * HID + 512 * c : i * HID + 512 * c + 512],
                            start=True,
                            stop=False,
                            tile_position=(0, 64 * half),
                        )
                    for kj, k in enumerate(K_ORDER):
                        for half in range(2):
                            c = 2 * g + half
                            nc.tensor.matmul(
                                ph[64 * half : 64 * half + 64, :],
                                zt_h[:, B * k : B * k + B],
                                w1t[k][:, 512 * c : 512 * c + 512],
                                start=False,
                                stop=(kj == KD - 1),
                                tile_position=(0, 64 * half),
                            )

                # tanh in two column-halves, then DMA-xbar transposes move h
                # into hid-major hT (frees the PE and the DVE): block (g,h,u)
                # of h_bm is exactly hT k-slice k = 8g + 4h + u.
                for g in range(2):
                    h_bm = hpool.tile([128, 512], mmdt, tag="h_bm")
                    for t in range(2):
                        nc.scalar.activation(
                            h_bm[:, 256 * t : 256 * t + 256],
                            phs[g][:, 256 * t : 256 * t + 256],
                            TANH,
                        )
                    for h in range(2):
                        for u in range(4):
                            k = 8 * g + 4 * h + u
                            nc.sync.dma_start_transpose(
                                hT[:, B * k : B * k + B],
                                h_bm[64 * h : 64 * h + 64, 128 * u : 128 * u + 128],
                            )

                # ---- mm2: f' = h @ W2', chunks of 512 over D, col-tiled ----
                pf = pf_pool.tile([128, 512], f32, tag="pf")
                for kj in range(KH):
                    k = kj  # hT slices land in plain k order from the DMA-Ts
                    for half in range(2):
                        nc.tensor.matmul(
                            pf[64 * half : 64 * half + 64, :],
                            hT[:, B * k : B * k + B],
                            w2t[k][:, 512 * half : 512 * half + 512],
                            start=(kj == 0),
                            stop=(kj == KH - 1),
                            tile_position=(0, 64 * half),
                        )

                # ---- transpose f' to d-major and update state ----
                # Per-128-block pipeline keeps the step boundary short: copy
                # block u (DVE) -> PE transpose u -> zt_h slices {u, u+4}
                # (fp16, straight from zt_old + f so the next step's mm1 is
                # not serialized behind the fp32 master update).
                f_bm = hpool.tile([128, 512], mmdt, tag="f_bm")
                for u in range(4):
                    nc.vector.tensor_copy(
                        f_bm[:, 128 * u : 128 * u + 128],
                        pf[:, 128 * u : 128 * u + 128],
                    )
                pt2 = pt_pool.tile([128, 512], mmdt, tag="pt")
                for u in range(4):
                    nc.tensor.matmul(
                        pt2[:, 128 * u : 128 * u + 128],
                        f_bm[:, 128 * u : 128 * u + 128],
                        ident_sb[:],
                        is_transpose=True,
                        start=True,
                        stop=True,
                    )
                zt_v = zt[:].rearrange("p (h u c) -> p h u c", h=2, u=4)
                zth_v = zt_h[:].rearrange("p (h u c) -> p h u c", h=2, u=4)
                pt2_v = pt2[:].rearrange("p (u h c) -> p h u c", u=4, h=2)
                if i < NSTEP - 1:
                    for u in range(4):
                        nc.vector.tensor_add(
                            zth_v[:, :, u : u + 1, :],
                            zt_v[:, :, u : u + 1, :],
                            pt2_v[:, :, u : u + 1, :],
                        )
                nc.vector.tensor_add(zt_v, zt_v, pt2_v)

        if repeat == 1:
            scan_body()
        else:
            with tc.For_i(0, repeat, 1) as _i:
                scan_body(_i)

        nc.sync.dma_start(zt_out[:], zt[:])

    nc.compile()
    return nc


def _pack_zT(shard):  # [B, D] -> [128, KD*B]
    return np.ascontiguousarray(
        shard.T.reshape(KD, 128, B).transpose(1, 0, 2).reshape(128, KD * B)
    )


def _unpack_zT(zt):  # [128, KD*B] -> [B, D]
    return zt.reshape(128, KD, B).transpose(1, 0, 2).reshape(D, B).T


def _host_inputs(z0, t, W1, b1, wt, W2, b2, np_dt):
    t = np.asarray(t, F32)
    t0s, t1s = t[:-1], t[1:]
    h_seg = (t1s - t0s) / 2.0  # N_STEPS_PER_SEG = 2
    step_ts = (t0s[:, None] + h_seg[:, None] * np.arange(2, dtype=F32)[None, :]).reshape(
        -1
    )
    step_hs = np.repeat(h_seg, 2)
    assert np.allclose(step_hs, step_hs[0]), "non-uniform Euler steps unsupported"
    scale = F32(step_hs[0])

    c = (scale * np.asarray(b2, F32)).astype(F32)  # [D]
    cW1 = (c.astype(np.float64) @ np.asarray(W1, np.float64)).astype(F32)  # [HID]
    brow = np.stack(
        [
            (np.asarray(b1, F32) + step_ts[i] * np.asarray(wt, F32) + i * cW1).astype(
                F32
            )
            for i in range(NSTEP)
        ]
    )  # [NSTEP, HID]
    brow_flat = brow.astype(np_dt).reshape(1, NSTEP * HID)

    w1p = np.ascontiguousarray(
        np.asarray(W1, F32)
        .reshape(KD, 128, HID)
        .transpose(1, 0, 2)
        .reshape(128, KD * HID)
    ).astype(np_dt)
    w2p = np.ascontiguousarray(
        (scale * np.asarray(W2, F32))
        .astype(F32)
        .reshape(KH, 128, D)
        .transpose(1, 0, 2)
        .reshape(128, KH * D)
    ).astype(np_dt)
    ident = np.eye(128, dtype=np_dt)
    ones = np.ones((1, B), np_dt)
    return brow_flat, w1p, w2p, ident, ones, c


def _make_in_maps(z0, t, W1, b1, wt, W2, b2, np_dt):
    z0 = np.asarray(z0, F32)
    brow_flat, w1p, w2p, ident, ones, c = _host_inputs(
        z0, t, W1, b1, wt, W2, b2, np_dt
    )
    in_maps = []
    for core in range(NCORES):
        shard = z0[core * B : (core + 1) * B]
        in_maps.append(
            {
                "zt_in": _pack_zT(shard),
                "w1": w1p,
                "w2": w2p,
                "brow": brow_flat,
                "ident": ident,
                "ones": ones,
            }
        )
    return in_maps, c


def run(z0, t, W1, b1, wt, W2, b2, trace=False, mm_dtype=MM_DTYPE):
    from concourse.bass_utils import run_bass_kernel_spmd

    np_dt = _np_dt(mm_dtype)
    in_maps, c = _make_in_maps(z0, t, W1, b1, wt, W2, b2, np_dt)
    nc = _build_program(mm_dtype=mm_dtype)
    res = run_bass_kernel_spmd(nc, in_maps, core_ids=list(range(NCORES)), trace=trace)

    outs = []
    for core in range(NCORES):
        z_shard = _unpack_zT(np.asarray(res.results[core]["zt_out"], F32))
        outs.append(z_shard)
    out = np.concatenate(outs, axis=0).astype(F32)
    out = out + (NSTEP * c)[None, :].astype(F32)
    return out.astype(F32), res


def kernel(z0, t, W1, b1, wt, W2, b2):
    out, _ = run(z0, t, W1, b1, wt, W2, b2, trace=False)
    return out


# revision 42
# speedup vs baseline: 2.5091x; 2.5091x over previous
"""Trainium2 Bass kernel for the NeuralODE problem.

Math (matching reference.py):
    20 Euler steps (10 segments x 2 steps, uniform dt => step size hi = 0.05):
        z_{i+1} = z_i + hi * ( tanh(z_i @ W1 + b1 + t_i*wt) @ W2 + b2 )

Device-side reformulation (per core, batch shard B=64):
    - Fold hi into W2:  W2' = hi * W2, c = hi * b2.
    - Keep the "state without accumulated c":  z'_i = z_i - i*c, so
        z'_{i+1} = z'_i + tanh(z'_i @ W1 + bias_i) @ W2'
      with bias_i = b1 + t_i*wt + i*(c @ W1)   (precomputed on host).
      Final output: z_20 = z'_20 + 20*c       (added on host).
    - State kept transposed (d-major) as zt[p, 64k+b] = z'[b, 128k+p] so it can be
      the stationary (lhsT) operand of orientation-B matmuls.
    - Both matmuls stream the (SBUF-resident) weights as the moving operand with
      N=512 chunks; the 64-wide batch stationary only fills half the PE columns,
      so two chunks run concurrently via tile_position col-tiling (0,0)/(0,64).
    - The per-step bias enters PSUM first through a K=1 ones-vector matmul.
    - Layout flips (batch-major PSUM result -> d/hid-major stationary for the next
      matmul) are done with full-128 PE transpose-mode matmuls against identity;
      one 128x128 transpose covers one 128-col block of both concurrent chunks.

Precision: matmul operands are float16 (1 cyc/col on the PE vs 4 for fp32;
fp32r would also be 1 cyc/col but its ISA rules require dst start_partition==0,
which forbids the (0,64) col-tiling). The master state zt stays fp32 in SBUF
and is re-cast to fp16 each step for the stationary operand; PSUM accumulation
is fp32 throughout, so only operand rounding (~5e-4) enters per step.

Sharding: pure data-parallel over batch (512 -> 8 x 64); weights replicated.
"""

import numpy as np

BS, D, HID = 512, 1024, 2048
NCORES = 8
B = BS // NCORES  # 64
NSTEP = 20
KD = D // 128  # 8 k-tiles for the D contraction
KH = HID // 128  # 16 k-tiles for the HID contraction
F32 = np.float32

MM_DTYPE = "float16"  # matmul operand dtype ("float16" | "bfloat16" | "float32")


def _np_dt(mm_dtype):
    return {"float16": np.float16, "bfloat16": None, "float32": np.float32}[mm_dtype]


def _build_program(mm_dtype=MM_DTYPE, repeat=1):
    import concourse.mybir as mybir
    from concourse import bacc
    from concourse.tile import TileContext

    nc = bacc.Bacc()
    f32 = mybir.dt.float32
    mmdt = getattr(mybir.dt, mm_dtype)
    TANH = mybir.ActivationFunctionType.Tanh

    zt_in = nc.dram_tensor("zt_in", [128, KD * B], f32, kind="ExternalInput")
    w1_d = nc.dram_tensor("w1", [128, KD * HID], mmdt, kind="ExternalInput")
    w2_d = nc.dram_tensor("w2", [128, KH * D], mmdt, kind="ExternalInput")
    # bias rows for the K=1 ones-matmul PSUM init, all steps
    brow_d = nc.dram_tensor("brow", [1, NSTEP * HID], mmdt, kind="ExternalInput")
    ones_d = nc.dram_tensor("ones", [1, B], mmdt, kind="ExternalInput")
    ident_d = nc.dram_tensor("ident", [128, 128], mmdt, kind="ExternalInput")
    zt_out = nc.dram_tensor("zt_out", [128, KD * B], f32, kind="ExternalOutput")

    # zt / hT / w1-rows / w2-rows keep their 128-tiles in PERMUTED order so
    # that transpose-block u of a [128,512] PE transpose lands contiguously at
    # cols [128u, 128u+128): position j holds original tile PERM[j] (per 8).
    # All device-side loops then run in plain position order; the host packs
    # and unpacks with the permutation.
    PERM = [0, 4, 1, 5, 2, 6, 3, 7]

    with (
        TileContext(nc) as tc,
        tc.tile_pool(name="const", bufs=1) as cpool,
        tc.tile_pool(name="weights", bufs=1) as wpool,
        tc.tile_pool(name="state", bufs=1) as spool,
        tc.tile_pool(name="work", bufs=2) as hpool,
        tc.tile_pool(name="psumh", bufs=3, space="PSUM") as ph_pool,
        tc.tile_pool(name="psumt", bufs=2, space="PSUM") as pt_pool,
        tc.tile_pool(name="psumf", bufs=2, space="PSUM") as pf_pool,
    ):
        ident_sb = cpool.tile([128, 128], mmdt, tag="ident")
        nc.sync.dma_start(ident_sb[:], ident_d[:])
        ones_sb = cpool.tile([1, B], mmdt, tag="ones")
        nc.sync.dma_start(ones_sb[:], ones_d[:])
        # all 20 g1-bias rows in one small upfront DMA (before the weights on
        # the same HWDGE FIFO)
        brow_sb = cpool.tile([1, NSTEP * HID], mmdt, tag="brow")
        nc.sync.dma_start(brow_sb[:], brow_d[:])

        zt = spool.tile([128, KD * B], f32, tag="zt")  # master z'_T  [128, 512] fp32
        nc.sync.dma_start(zt[:], zt_in[:])
        zt_h = spool.tile([128, KD * B], mmdt, tag="zt_h")  # fp16 copy for the PE
        hT = spool.tile([128, KH * B], mmdt, tag="hT")  # tanh'd h, hid-major [128,1024]

        # per-position weight tiles (host already permuted the row-tiles),
        # DMA-issued in consumption order so step-0 matmuls can start as soon
        # as their own slice has landed
        w1t = []
        for k in range(KD):
            w = wpool.tile([128, HID], mmdt, tag=f"w1_{k}")
            nc.sync.dma_start(w[:], w1_d[:, k * HID : (k + 1) * HID])
            w1t.append(w)
        w2t = []
        for k in range(KH):
            w = wpool.tile([128, D], mmdt, tag=f"w2_{k}")
            nc.sync.dma_start(w[:], w2_d[:, k * D : (k + 1) * D])
            w2t.append(w)

        def scan_body(_iv=None):
            for i in range(NSTEP):
                if i == 0:
                    nc.vector.tensor_copy(zt_h[:], zt[:])

                # ---- mm1: h_pre = z @ W1 + bias_i (K=1 ones mm inits PSUM) ----
                phs = []
                for g in range(2):
                    ph = ph_pool.tile([128, 512], f32, tag="ph")
                    phs.append(ph)
                    for half in range(2):
                        c = 2 * g + half
                        nc.tensor.matmul(
                            ph[64 * half : 64 * half + 64, :],
                            ones_sb[:1, :],
                            brow_sb[:1, i * HID + 512 * c : i * HID + 512 * c + 512],
                            start=True,
                            stop=False,
                            tile_position=(0, 64 * half),
                        )
                    for kj in range(KD):
                        for half in range(2):
                            c = 2 * g + half
                            nc.tensor.matmul(
                                ph[64 * half : 64 * half + 64, :],
                                zt_h[:, B * kj : B * kj + B],
                                w1t[kj][:, 512 * c : 512 * c + 512],
                                start=False,
                                stop=(kj == KD - 1),
                                tile_position=(0, 64 * half),
                            )

                # tanh in two column-halves so the transposes can start early;
                # hT copied per 128-block so mm2's k-slices unblock in K_ORDER.
                for g in range(2):
                    h_bm = hpool.tile([128, 512], mmdt, tag="h_bm")
                    for t in range(2):
                        nc.scalar.activation(
                            h_bm[:, 256 * t : 256 * t + 256],
                            phs[g][:, 256 * t : 256 * t + 256],
                            TANH,
                        )
                    pt = pt_pool.tile([128, 512], mmdt, tag="pt")
                    for u in range(4):
                        nc.tensor.matmul(
                            pt[:, 128 * u : 128 * u + 128],
                            h_bm[:, 128 * u : 128 * u + 128],
                            ident_sb[:],
                            is_transpose=True,
                            start=True,
                            stop=True,
                        )
                    # hT keeps tiles in transpose-block order -> plain copies
                    for u in range(4):
                        nc.vector.tensor_copy(
                            hT[:, 512 * g + 128 * u : 512 * g + 128 * u + 128],
                            pt[:, 128 * u : 128 * u + 128],
                        )

                # ---- mm2: f' = h @ W2', chunks of 512 over D, col-tiled ----
                pf = pf_pool.tile([128, 512], f32, tag="pf")
                for kj in range(KH):
                    for half in range(2):
                        nc.tensor.matmul(
                            pf[64 * half : 64 * half + 64, :],
                            hT[:, B * kj : B * kj + B],
                            w2t[kj][:, 512 * half : 512 * half + 512],
                            start=(kj == 0),
                            stop=(kj == KH - 1),
                            tile_position=(0, 64 * half),
                        )

                # ---- transpose f' to d-major and update state ----
                # Per-128-block pipeline keeps the step boundary short: copy
                # block u (DVE) -> PE transpose u -> zt_h slices {u, u+4}
                # (fp16, straight from zt_old + f so the next step's mm1 is
                # not serialized behind the fp32 master update).
                f_bm = hpool.tile([128, 512], mmdt, tag="f_bm")
                for u in range(4):
                    nc.vector.tensor_copy(
                        f_bm[:, 128 * u : 128 * u + 128],
                        pf[:, 128 * u : 128 * u + 128],
                    )
                pt2 = pt_pool.tile([128, 512], mmdt, tag="pt")
                for u in range(4):
                    nc.tensor.matmul(
                        pt2[:, 128 * u : 128 * u + 128],
                        f_bm[:, 128 * u : 128 * u + 128],
                        ident_sb[:],
                        is_transpose=True,
                        start=True,
                        stop=True,
                    )
                # zt keeps tiles in transpose-block order -> contiguous adds
                if i < NSTEP - 1:
                    for u in range(4):
                        nc.vector.tensor_add(
                            zt_h[:, 128 * u : 128 * u + 128],
                            zt[:, 128 * u : 128 * u + 128],
                            pt2[:, 128 * u : 128 * u + 128],
                        )
                nc.vector.tensor_add(zt[:], zt[:], pt2[:])

        if repeat == 1:
            scan_body()
        else:
            with tc.For_i(0, repeat, 1) as _i:
                scan_body(_i)

        nc.sync.dma_start(zt_out[:], zt[:])

    nc.compile()
    return nc


PERM = [0, 4, 1, 5, 2, 6, 3, 7]  # position j holds original 128-tile PERM[j]


def _pack_zT(shard):  # [B, D] -> [128, KD*B], d-tiles in PERM position order
    arr = shard.T.reshape(KD, 128, B)[PERM]
    return np.ascontiguousarray(arr.transpose(1, 0, 2).reshape(128, KD * B))


def _unpack_zT(zt):  # [128, KD*B] -> [B, D]
    arr = zt.reshape(128, KD, B).transpose(1, 0, 2)  # [position, 128, B]
    inv = np.argsort(PERM)
    return arr[inv].reshape(D, B).T


def _host_inputs(z0, t, W1, b1, wt, W2, b2, np_dt):
    t = np.asarray(t, F32)
    t0s, t1s = t[:-1], t[1:]
    h_seg = (t1s - t0s) / 2.0  # N_STEPS_PER_SEG = 2
    step_ts = (t0s[:, None] + h_seg[:, None] * np.arange(2, dtype=F32)[None, :]).reshape(
        -1
    )
    step_hs = np.repeat(h_seg, 2)
    assert np.allclose(step_hs, step_hs[0]), "non-uniform Euler steps unsupported"
    scale = F32(step_hs[0])

    c = (scale * np.asarray(b2, F32)).astype(F32)  # [D]
    cW1 = (c.astype(np.float64) @ np.asarray(W1, np.float64)).astype(F32)  # [HID]
    brow = np.stack(
        [
            (np.asarray(b1, F32) + step_ts[i] * np.asarray(wt, F32) + i * cW1).astype(
                F32
            )
            for i in range(NSTEP)
        ]
    )  # [NSTEP, HID]
    brow_flat = brow.astype(np_dt).reshape(1, NSTEP * HID)

    w1p = np.ascontiguousarray(
        np.asarray(W1, F32)
        .reshape(KD, 128, HID)[PERM]  # d-tiles in zt position order
        .transpose(1, 0, 2)
        .reshape(128, KD * HID)
    ).astype(np_dt)
    w2perm = [8 * g + j for g in range(2) for j in PERM]  # hT position order
    w2p = np.ascontiguousarray(
        (scale * np.asarray(W2, F32))
        .astype(F32)
        .reshape(KH, 128, D)[w2perm]
        .transpose(1, 0, 2)
        .reshape(128, KH * D)
    ).astype(np_dt)
    ident = np.eye(128, dtype=np_dt)
    ones = np.ones((1, B), np_dt)
    return brow_flat, w1p, w2p, ident, ones, c


def _make_in_maps(z0, t, W1, b1, wt, W2, b2, np_dt):
    z0 = np.asarray(z0, F32)
    brow_flat, w1p, w2p, ident, ones, c = _host_inputs(
        z0, t, W1, b1, wt, W2, b2, np_dt
    )
    in_maps = []
    for core in range(NCORES):
        shard = z0[core * B : (core + 1) * B]
        in_maps.append(
            {
                "zt_in": _pack_zT(shard),
                "w1": w1p,
                "w2": w2p,
                "brow": brow_flat,
                "ident": ident,
                "ones": ones,
            }
        )
    return in_maps, c


def run(z0, t, W1, b1, wt, W2, b2, trace=False, mm_dtype=MM_DTYPE):
    from concourse.bass_utils import run_bass_kernel_spmd

    np_dt = _np_dt(mm_dtype)
    in_maps, c = _make_in_maps(z0, t, W1, b1, wt, W2, b2, np_dt)
    nc = _build_program(mm_dtype=mm_dtype)
    res = run_bass_kernel_spmd(nc, in_maps, core_ids=list(range(NCORES)), trace=trace)

    outs = []
    for core in range(NCORES):
        z_shard = _unpack_zT(np.asarray(res.results[core]["zt_out"], F32))
        outs.append(z_shard)
    out = np.concatenate(outs, axis=0).astype(F32)
    out = out + (NSTEP * c)[None, :].astype(F32)
    return out.astype(F32), res


def kernel(z0, t, W1, b1, wt, W2, b2):
    out, _ = run(z0, t, W1, b1, wt, W2, b2, trace=False)
    return out
